# revision 56
# baseline (speedup 1.0000x reference)
"""DeepFourierTransform kernel for Trainium2 (8 NeuronCores, data-parallel).

Problem:
  x [4096, 4096] f32 -> sliding windows (31 per row, size 256, hop 128)
  cos_feat = cos(win @ w_cos.T + b_cos)   [B, 31, 512]
  sin_feat = sin(win @ w_sin.T + b_sin)   [B, 31, 512]
  out = concat(cos,sin) @ w_out.T + b_out, mean over windows, log_softmax
  -> [4096, 4] f32

Strategy (per core, batch shard of 512 rows; cost-model makespan ~135us,
ACT-bound at ~89% occupancy vs its ~121us busy floor):
  - Host: cast x to bf16 and pre-transpose to xT [4096, 512]; weights packed
    combo-major [128, combo, half, 128] so combo 0's 64KB slice can load
    before the bulk; fold pi/2 into the cos bias (cos x = sin(x + pi/2));
    fold the 1/31 window-mean into w_out.
  - Device layout is feature-major: psum[m_tile=128p, b=512f]. Window w's
    linear term = wT[half0].T @ xT[chunk w] + wT[half1].T @ xT[chunk w+1],
    accumulated in PSUM (K=256 as 2 bf16 matmuls of K=128, N=512).
  - 8 "combos" = (cos m0..3, sin m0..3) feature tiles of 128. Window blocks of
    up to 4 -> psum tile [128, 4, 512] = 4 banks, double-buffered (8 banks);
    one Sin activation with fused per-partition bias reads a whole block
    (FD up to 2048) from PSUM and writes bf16 feat to SBUF.
  - DVE bf16 (2x-mode) tree-adds accumulate feat over windows into
    acc [128, 8, 512].
  - Final projection (lhsT = acc slices, rhs = w_out.T/31 [128, 4])
    accumulates into 4 one-bank fft psum tiles; + b_out, then a batched
    log_softmax along the free dim (no max-shift: |z| <= ~3); single DMA out.
  - Exp/Ln are steered to the shared natural_log_exp table set (one tail
    table load); a warmup Sin pulls the trig table load to t~0; dummy
    matmuls during the initial DMA wait pre-warm the PE clock (HAM).
"""

import numpy as np
import ml_dtypes

import concourse.bass as bass
import concourse.bacc as bacc
import concourse.mybir as mybir
import concourse.tile as tile
from concourse.bass_utils import run_bass_kernel_spmd

BF16 = mybir.dt.bfloat16
F32 = mybir.dt.float32

N_CORES = 8
B = 4096
B_LOCAL = B // N_CORES          # 512
SEQ = 4096
P = 128
NCHUNK = SEQ // P               # 32
NWIN = 31
M = 512                         # features per trig branch
NCOMBO = 8                      # 4 cos m-tiles + 4 sin m-tiles
OUT_DIM = 4
# window blocks: (first_window, n_windows): seven 4-window blocks (psum tile
# = 4 banks, double-buffered = all 8 banks) + a 3-window tail block. This is
# the minimal activation-instruction count (8 blocks x 8 combos); the PE
# warmup + early small DMA groups cover the startup that a bigger first
# block would otherwise pay.
BLOCKS = [(4 * i, 4) for i in range(7)] + [(28, 3)]

_CACHED_NC = None
NWARM = 6  # PE/HAM warmup matmuls issued during the initial DMA wait


class _Bacc(bacc.Bacc):
    """Bacc with a curated activation-table list: Exp/Ln resolve to the shared
    natural_log_exp_and_others set (one tail table load instead of two).
    Positions stay canonical so emitted act_func_set_ids remain valid."""

    def insert_act_table_loads(self):
        import bass_rust as _br
        from concourse.hw_specs import get_activation_tables

        has_activation = any(
            isinstance(i, mybir.InstActivation)
            for b in self.main_func.blocks
            for i in b.instructions
        )
        if not has_activation:
            return
        act = mybir.ActivationFunctionType
        tables = list(get_activation_tables(self.m.arch).items())
        names = [n for n, _ in tables]
        if "natural_log_exp_and_others" in names:
            keep = names.index("natural_log_exp_and_others")
            tables = [
                (
                    n,
                    fns
                    if i == keep
                    else {f for f in fns if f not in (act.Exp, act.Ln)},
                )
                for i, (n, fns) in enumerate(tables)
            ]
        _br.insert_act_table_loads(self, tables)


def _build_nc():
    nc = _Bacc()
    act = mybir.ActivationFunctionType
    alu = mybir.AluOpType

    x = nc.dram_tensor("x", [SEQ, B_LOCAL], BF16, kind="ExternalInput")  # xT
    wt = nc.dram_tensor("wt", [P, NCOMBO, 2, P], BF16, kind="ExternalInput")
    bias = nc.dram_tensor("bias", [P, NCOMBO], F32, kind="ExternalInput")
    wot = nc.dram_tensor("wot", [P, NCOMBO, OUT_DIM], BF16, kind="ExternalInput")
    bot = nc.dram_tensor("bot", [P, OUT_DIM], F32, kind="ExternalInput")
    y = nc.dram_tensor("y", [B_LOCAL, OUT_DIM], F32, kind="ExternalOutput")

    with tile.TileContext(nc) as tc:
        with (
            tc.tile_pool(name="consts", bufs=1) as consts,
            tc.tile_pool(name="xt", bufs=1) as xtp,
            tc.tile_pool(name="feat", bufs=20) as featp,
            tc.tile_pool(name="tmp", bufs=6) as tmpp,
            tc.tile_pool(name="accp", bufs=1) as accp,
            tc.tile_pool(name="tail", bufs=2) as tailp,
        ):
            # ---- warmup: pull the Sin table load to t~0 on ACT ----
            warm = consts.tile([P, 1], F32)
            nc.vector.memset(warm, 0.0)
            warm2 = consts.tile([P, 1], F32)
            nc.scalar.activation(warm2, warm, act.Sin, scale=1.0)
            # PE/HAM warmup operand: dummy matmuls during the initial DMA wait
            # keep the PE busy so the first real matmuls run at full clock.
            wrm = consts.tile([P, B_LOCAL], BF16)
            nc.vector.memset(wrm, 0.0)

            # ---- constants: wt is on the MM critical path -> dense + HWDGE
            # first; small consts go via SWDGE to stay off the HWDGE queue ----
            # combo-major weights so combo 0's 64KB slice loads first
            wt_sb = consts.tile([P, NCOMBO, 2, P], BF16)
            nc.sync.dma_start(wt_sb[:, 0], wt[:, 0])
            bias_sb = consts.tile([P, NCOMBO], F32)
            nc.gpsimd.dma_start(bias_sb, bias[:, :])
            wot_sb = consts.tile([P, NCOMBO, OUT_DIM], BF16)
            nc.gpsimd.dma_start(wot_sb, wot[:, :, :])
            bot_sb = consts.tile([P, OUT_DIM], F32)
            nc.gpsimd.dma_start(bot_sb, bot[:, :])

            # ---- x (already transposed on host): small groups first for
            # startup latency, larger groups later for DMA efficiency ----
            GROUP_SIZES = [2, 2, 2, 4, 4, 4, 4, 4, 4, 2]
            assert sum(GROUP_SIZES) == NCHUNK
            xts = []  # per chunk: (tile, index within tile)
            k0 = 0
            for g, gsz in enumerate(GROUP_SIZES):
                if g == 3:  # after chunks 0-4: combo 1, then combos 2-7
                    nc.sync.dma_start(wt_sb[:, 1], wt[:, 1])
                    nc.sync.dma_start(wt_sb[:, 2:], wt[:, 2:])
                t = xtp.tile([P, gsz, B_LOCAL], BF16, tag=f"xt{g}")
                nc.sync.dma_start(
                    t,
                    x[k0 * P : (k0 + gsz) * P, :].rearrange("(k p) b -> p k b", p=P),
                )
                for i in range(gsz):
                    xts.append((t, i))
                k0 += gsz

            def xchunk(k):
                t, i = xts[k]
                return t[:, i, :]

            acc = accp.tile([P, NCOMBO, B_LOCAL], BF16)

            NBT = B_LOCAL // P

            def mm_block(ps, c, w0, nw):
                for wi in range(nw):
                    w = w0 + wi
                    nc.tensor.matmul(
                        ps[:, wi, :],
                        lhsT=wt_sb[:, c, 0, :],
                        rhs=xchunk(w),
                        start=True,
                        stop=False,
                    )
                    nc.tensor.matmul(
                        ps[:, wi, :],
                        lhsT=wt_sb[:, c, 1, :],
                        rhs=xchunk(w + 1),
                        start=False,
                        stop=True,
                    )

            def sin_block(ps, c, nw):
                ft = featp.tile([P, 4, B_LOCAL], BF16, tag="ft")
                nc.scalar.activation(
                    ft[:, :nw, :],
                    ps[:, :nw, :],
                    act.Sin,
                    bias=bias_sb[:, c : c + 1],
                    scale=1.0,
                )
                return ft

            # ---- main loop: all blocks except the last ----
            with tc.tile_pool(name="psum", bufs=2, space="PSUM") as psump:
                if NWARM:
                    wps = psump.tile([P, 4, B_LOCAL], F32, tag="ps", name="wps")
                    for _ in range(NWARM):
                        nc.tensor.matmul(
                            wps[0:1, 0, :],
                            lhsT=wrm[:, 0:1],
                            rhs=wrm,
                            start=True,
                            stop=True,
                        )
                for bi, (w0, nw) in enumerate(BLOCKS):
                    for c in range(NCOMBO):
                        ps = psump.tile([P, 4, B_LOCAL], F32, tag="ps")
                        mm_block(ps, c, w0, nw)
                        ft = sin_block(ps, c, nw)
                        # reduce ft[:, :nw, :] into `red`, then init/accumulate
                        if nw == 1:
                            red = ft[:, 0, :]
                        elif nw == 2:
                            red = tmpp.tile([P, B_LOCAL], BF16, tag="s", name="s2")
                            nc.vector.tensor_add(red, ft[:, 0, :], ft[:, 1, :])
                        elif nw == 3:
                            s = tmpp.tile([P, B_LOCAL], BF16, tag="s", name="s3a")
                            nc.vector.tensor_add(s, ft[:, 0, :], ft[:, 1, :])
                            red = tmpp.tile([P, B_LOCAL], BF16, tag="s", name="s3b")
                            nc.vector.tensor_add(red, s, ft[:, 2, :])
                        else:  # nw == 4
                            pr = tmpp.tile([P, 2, B_LOCAL], BF16, tag="pr")
                            nc.vector.tensor_add(pr, ft[:, 0:2, :], ft[:, 2:4, :])
                            red = tmpp.tile([P, B_LOCAL], BF16, tag="s", name="s4")
                            nc.vector.tensor_add(red, pr[:, 0, :], pr[:, 1, :])
                        if bi == 0:
                            nc.vector.tensor_copy(acc[:, c, :], red)
                        else:
                            nc.vector.tensor_add(acc[:, c, :], acc[:, c, :], red)

            # ---- final projection (after all blocks) ----
            z_all = tailp.tile([P, NBT, OUT_DIM], F32, tag="z")
            with tc.tile_pool(name="pfft", bufs=NBT, space="PSUM") as pfft:
                for bt in range(NBT):
                    pf = pfft.tile([P, OUT_DIM], F32, tag="pf", name=f"pf{bt}")
                    for c in range(NCOMBO):
                        nc.tensor.matmul(
                            pf,
                            lhsT=acc[:, c, bt * P : (bt + 1) * P],
                            rhs=wot_sb[:, c, :],
                            start=(c == 0),
                            stop=(c == NCOMBO - 1),
                        )
                    nc.vector.tensor_add(z_all[:, bt, :], pf, bot_sb)
            # ---- log_softmax tail (batched; |z| <= ~3 so no max-shift
            # needed: out = z - ln(sum(exp(z))) is exact-enough in fp32) ----
            e = tailp.tile([P, NBT, OUT_DIM], F32, tag="e")
            nc.scalar.activation(e, z_all, act.Exp)
            ssum = tailp.tile([P, NBT], F32, tag="ss")
            nc.vector.reduce_sum(ssum, e, axis=mybir.AxisListType.X)
            ls = tailp.tile([P, NBT], F32, tag="ls")
            nc.scalar.activation(ls, ssum, act.Ln)
            o = tailp.tile([P, NBT, OUT_DIM], F32, tag="o")
            nc.vector.tensor_tensor(
                o,
                z_all,
                ls[:, :, None].to_broadcast([P, NBT, OUT_DIM]),
                mybir.AluOpType.subtract,
            )
            nc.sync.dma_start(y.rearrange("(bt p) o -> p bt o", p=P), o)

    if not nc.is_finalized():
        nc.finalize()
    return nc


def _get_nc():
    global _CACHED_NC
    if _CACHED_NC is None:
        _CACHED_NC = _build_nc()
    return _CACHED_NC


def _make_in_maps(x, w_cos, b_cos, w_sin, b_sin, w_out, b_out):
    bf = ml_dtypes.bfloat16
    x = np.asarray(x)
    w_cos, w_sin = np.asarray(w_cos), np.asarray(w_sin)
    b_cos, b_sin = np.asarray(b_cos), np.asarray(b_sin)
    w_out, b_out = np.asarray(w_out), np.asarray(b_out)
    # weights: [p, half, cos m | sin m], dense bf16 (wt[p,h,m] = wT[h*128+p, m])
    wt = np.concatenate([w_cos.T, w_sin.T], axis=1).reshape(2, P, NCOMBO, P)
    wt = np.ascontiguousarray(wt.transpose(1, 2, 0, 3)).astype(bf)
    # per-combo per-partition biases; fold pi/2 into cos (cos x = sin(x+pi/2))
    bias = np.empty((P, NCOMBO), np.float32)
    for mt in range(4):
        bias[:, mt] = b_cos[mt * P : (mt + 1) * P] + np.float32(np.pi / 2)
        bias[:, 4 + mt] = b_sin[mt * P : (mt + 1) * P]
    # w_out.T with 1/31 mean folded in, chunked to [p, combo, o]
    wot = (w_out.T.astype(np.float64) / NWIN).astype(np.float32)
    wot = wot.reshape(NCOMBO, P, OUT_DIM).transpose(1, 0, 2).astype(bf)
    bot = np.broadcast_to(b_out.astype(np.float32), (P, OUT_DIM)).copy()

    in_maps = []
    for c in range(N_CORES):
        xs = x[c * B_LOCAL : (c + 1) * B_LOCAL, :]
        xt = np.ascontiguousarray(xs.T.astype(bf))  # [4096, 512]
        in_maps.append(
            {"x": xt, "wt": wt, "bias": bias, "wot": wot, "bot": bot}
        )
    return in_maps


def run(inputs, trace=False, trace_cores=None):
    """Run the kernel; returns (y_full [4096,4] f32, BassKernelResults).

    Retries once on transient device errors (the terminal occasionally
    reports NRT_EXEC_UNIT_UNRECOVERABLE after a prior crashed session and
    recovers on the next attempt)."""
    import time

    nc = _get_nc()
    in_maps = _make_in_maps(**inputs)
    last_err = None
    for attempt in range(3):
        try:
            res = run_bass_kernel_spmd(
                nc,
                in_maps,
                core_ids=list(range(N_CORES)),
                trace=trace,
                trace_cores=trace_cores,
            )
            y = np.concatenate([r["y"] for r in res.results], axis=0)
            return y, res
        except Exception as e:  # transient device wedge -> retry
            last_err = e
            if "UNRECOVERABLE" not in str(e) and "UNAVAILABLE" not in str(e):
                raise
            time.sleep(2.0)
    raise last_err


def kernel(**inputs):
    y, _ = run(inputs, trace=False)
    return y
